# revision 1
# baseline (speedup 1.0000x reference)
"""NetVLAD Trainium2 kernel (Bass/Tile), data-parallel over batch on 8 cores.

Math (per batch b):
    x_hat = x / ||x||_2(channel)                    (B, D, H*W), D=512, N=1200
    logits = conv_w @ x_hat                         (K, N), K=64
    a = softmax_K(logits)
    vlad[k,d] = sum_n a[k,n] * x_hat[d,n] - (sum_n a[k,n]) * c[k,d]
    vlad = l2norm_rows(vlad); out = l2norm(flatten(vlad))   # == vlad_rows/8

Device-side structure (v3):
  - x stays in natural (D-major) layout for the PE: per (n-chunk, d-chunk)
    a fp32r transpose plus a fp32r logits matmul (accumulating over the 4
    d-chunks in PSUM).  The transposed x is evicted PSUM->SBUF with a
    fp32->bf16 cast so the aggregation matmul runs in bf16.
  - softmax tail is batched into whole-batch single ops to dodge the
    ~150-350ns per-instruction engine overheads: one DVE pre-scale
    (logits * sinv, broadcast), one big ACT exp over [P, 10*64], one
    batched DVE denominator reduce, one batched DVE a'-scale.  Junk lanes
    (partitions >= 48 of the last 48-wide chunk) flow through harmlessly
    and are never consumed.
  - normalization scale is folded into the softmax weights instead of x:
        a'[n,k] = a[n,k] / s[n]   =>   vlad term 1 contracts a' with RAW x.
  - rsqrt is computed as exp(-0.5*ln(.)) so the single ACT table set
    natural_log_exp_and_others covers every ScalarE op in the kernel.
  - asum[k] = sum_n a[k,n] obtained by streaming the norm column s (stored
    as 2 extra bf16 columns of the transposed-x tile) through the
    aggregation weights.
  - Pool (gpsimd, the Q7 DSP block) cannot touch PSUM and is ~15x slower
    per column than DVE, so it only gets tiny SBUF ops (memsets, s-copies,
    comb, gkn) and the output DMA triggers.
  - PSUM is four immortal tensors with manual region rotation (8 banks
    exactly) because pool slot re-acquisition joins producer+consumer sems
    into >1 sync wait on a Matmult, which walrus's S3_LW struct cannot
    encode.  Each batch runs three warm matmuls: one observes the first x
    DMA part, one absorbs the WAR of the prescale read of this logits
    parity bank (two batches ago), one observes the second x DMA part
    (the load is split by pixel range so chunk 0 starts ~4x earlier).
  - sum-of-squares passes for chunks 0-3 are deferred to iterations 6-9 so
    the early iterations keep the PSUM casts at the engine queue heads
    (the PE's 2-deep region rotation otherwise starves).

  Hard-won HW facts baked into this design: the PE sustains only ~1.2 GHz
  under load (2.4 GHz appears only in short post-idle bursts); fp32r
  matmuls with moving dim < 256 run at 4 cycles/row; matmul outputs are
  capped at 512 PSUM columns and must be fp32; DVE 2x/4x 16-bit modes do
  not engage for accumulate ops; free-axis tensor_reduce is DVE-only.
"""

import numpy as np

import concourse.bass as bass
import concourse.mybir as mybir
from concourse import bacc
import concourse.tile as tile
from concourse.bass_utils import run_bass_kernel_spmd
from concourse.masks import make_identity
from concourse.tile_rust import add_dep_helper

F32 = mybir.dt.float32
F32R = mybir.dt.float32r
BF16 = mybir.dt.bfloat16
ALU = mybir.AluOpType
ACTF = mybir.ActivationFunctionType

P = 128
BPC = 8            # batches per core
D = 512
N = 1200
K = 64
DCH = D // P       # 4 d-chunks
NCHUNKS = [(j * P, min(P, N - j * P)) for j in range((N + P - 1) // P)]  # 10
NJ = len(NCHUNKS)
DP2 = D + 2        # xt columns: [x^T | s s]
FWW = 256          # fused moving-operand width
N4 = 256           # first-part pixel count of the split x DMA (chunks 0-1)
N8 = 512           # second-part end (chunks 2-3)
LN_EIGHTH = float(np.log(0.125))

# per-chunk engine assignment: "a"=ACT, "v"=DVE.  Casts alternate engines
# so consecutive region evictions (the PE's WAR critical path) pipeline.
CAST_ENG = "a a v a a v a a v a".split()   # PSUM->SBUF x^T cast
SQ_ENG = "v v a v v a v v a v".split()     # sum-of-squares pass
# which previous-batch aggregation chunks run after chunk j's fillers
AGG_SCHED = {3: [0, 1, 2], 4: [3, 4], 5: [5, 6], 6: [7, 8], 7: [9]}


def _emit(nc):
    x = nc.dram_tensor("x", (BPC, D, N), F32R, kind="ExternalInput")
    wt = nc.dram_tensor("wt", (D, K), F32R, kind="ExternalInput")
    cent = nc.dram_tensor("cent", (K, D), F32, kind="ExternalInput")
    out = nc.dram_tensor("out", (BPC, K, D), F32, kind="ExternalOutput")

    with tile.TileContext(nc) as tc:
        with (
            tc.tile_pool(name="const", bufs=1) as const,
            tc.tile_pool(name="xnat", bufs=6) as xnat_pool,
            tc.tile_pool(name="xtsb", bufs=2) as xt_pool,
            tc.tile_pool(name="softmax", bufs=2) as sm_pool,
            tc.tile_pool(name="smalls", bufs=2) as smalls,
            tc.tile_pool(name="epilog", bufs=2) as ep_pool,
            tc.tile_pool(name="psum", bufs=1, space="PSUM") as psum,
        ):
            identf = const.tile([P, P], F32)
            make_identity(nc, identf)
            ident = const.tile([P, P], F32R)
            nc.vector.tensor_copy(ident, identf)
            wt_sb = const.tile([P, DCH, K], F32R)
            nc.sync.dma_start(wt_sb, wt[:, :].rearrange("(a p) k -> p a k", p=P))
            cent_sb = const.tile([K, D], F32)
            nc.sync.dma_start(cent_sb, cent[:, :])
            ln8 = const.tile([K, 1], F32)
            nc.gpsimd.memset(ln8, LN_EIGHTH)
            # never-read junk outputs for square-accumulate passes
            sqj = const.tile([P, D], BF16)
            sqj2 = const.tile([P, D], BF16)
            sqj3 = const.tile([K, D], BF16)

            # Immortal PSUM (8 banks exactly): transposed-x double buffer,
            # logits parity banks, vlad, asum.
            xtp = psum.tile([P, 2, D], F32)      # 2 banks, region j%2
            lg = psum.tile([P, 2, 1024], F32)    # 4 banks, parity b%2
            vl = psum.tile([K, D], F32)          # 1 bank
            asum = psum.tile([K, 2], F32)        # 1 bank

            # PE pre-observes the DVE-produced ident so the first real
            # transpose only carries the wt DMA wait.
            nc.tensor.matmul(
                lg[0:2, 0, 1020:1022],
                ident[:, 0:2],
                ident[:, 0:2],
                start=True,
                stop=True,
                skip_group_check=True,
            )

            state = {}

            def tail_pieces(b):
                """Softmax tail of batch b, as per-chunk filler closures.

                Emitted interleaved into phase1(b+1)'s chunk loop so these
                ops don't sit ahead of b+1's PSUM evictions in the in-order
                ACT/DVE queues (which would stall the PE on region WAR).
                """
                st = state[b]
                ss, xt = st["ss"], st["xt"]

                def t0():  # ACT: sinv = exp(-0.5*ln(ss))
                    lss = smalls.tile([P, NJ], F32, tag="lss")
                    nc.scalar.activation(lss, ss, ACTF.Ln)
                    sinv = smalls.tile([P, NJ], F32, tag="sinv")
                    nc.scalar.activation(sinv, lss, ACTF.Exp, scale=-0.5)
                    st["sinv"] = sinv

                def t1():  # DVE: prescale logits; Pool: s into xt tail cols
                    sinv = st["sinv"]
                    lgv = lg[:, st["par"], 0 : NJ * K].rearrange(
                        "p (j k) -> p j k", j=NJ
                    )
                    lgsc = sm_pool.tile([P, NJ, K], BF16, tag="lgsc")
                    nc.vector.tensor_tensor(
                        lgsc,
                        lgv,
                        sinv.unsqueeze(-1).to_broadcast((P, NJ, K)),
                        ALU.mult,
                    )
                    st["lgsc"] = lgsc
                    s = smalls.tile([P, NJ], F32, tag="s")
                    nc.gpsimd.tensor_tensor(s, ss, sinv, ALU.mult)
                    nc.gpsimd.tensor_copy(xt[:, :, D], s)
                    nc.gpsimd.tensor_copy(xt[:, :, D + 1], s)

                def t2():  # ACT: one big exp
                    expt = sm_pool.tile([P, NJ, K], BF16, tag="expt")
                    nc.scalar.activation(expt, st["lgsc"], ACTF.Exp)
                    st["expt"] = expt

                def t3():  # DVE: denominators; Pool: comb
                    den = smalls.tile([P, NJ], F32, tag="den")
                    nc.vector.tensor_reduce(
                        den, st["expt"], axis=mybir.AxisListType.X, op=ALU.add
                    )
                    rden = smalls.tile([P, NJ], F32, tag="rden")
                    nc.vector.reciprocal(rden, den)
                    comb = smalls.tile([P, NJ], F32, tag="comb")
                    nc.gpsimd.tensor_tensor(comb, rden, st["sinv"], ALU.mult)
                    st["comb"] = comb

                def t4():  # DVE: a' = expt * comb (bf16)
                    atp = sm_pool.tile([P, NJ, K], BF16, tag="atp")
                    nc.vector.tensor_tensor(
                        atp,
                        st["expt"],
                        st["comb"].unsqueeze(-1).to_broadcast((P, NJ, K)),
                        ALU.mult,
                    )
                    st["atp"] = atp

                return [t0, t1, t2, t3, t4]

            def phase2_pieces(b):
                """Epilog of batch b (vlad normalization), as fillers."""
                st = state[b]

                def p0():  # DVE: negd = asum*c - vlad
                    negd = ep_pool.tile([K, D], F32, tag="negd")
                    nc.vector.scalar_tensor_tensor(
                        out=negd,
                        in0=cent_sb,
                        scalar=asum[:, 0:1],
                        in1=vl[:, :],
                        op0=ALU.mult,
                        op1=ALU.subtract,
                    )
                    st["negd"] = negd

                def p1():  # ACT: row sum of squares
                    ssk = ep_pool.tile([K, 1], F32, tag="ssk")
                    nc.scalar.activation(
                        sqj3[:, :], st["negd"], ACTF.Square, accum_out=ssk
                    )
                    st["ssk"] = ssk

                def p2():  # ACT: gk = (1/8)*rsqrt(ssk)
                    lssk = ep_pool.tile([K, 1], F32, tag="lssk")
                    nc.scalar.activation(lssk, st["ssk"], ACTF.Ln)
                    gk = ep_pool.tile([K, 1], F32, tag="gk")
                    nc.scalar.activation(
                        gk, lssk, ACTF.Exp, scale=-0.5, bias=ln8
                    )
                    st["gk"] = gk

                def p3():  # DVE: scale rows; Pool: output DMA
                    ot = ep_pool.tile([K, D], F32, tag="ot")
                    nc.vector.tensor_scalar(
                        out=ot,
                        in0=st["negd"],
                        scalar1=st["gk"],
                        scalar2=-1.0,
                        op0=ALU.mult,
                        op1=ALU.mult,
                    )
                    nc.gpsimd.dma_start(out[b, :, :], ot)
                    state.pop(b)

                return [p0, p1, p2, p3]

            def phase1(b, fillers):
                par = b % 2
                xb = xnat_pool.tile([P, DCH, N], F32R, tag="xb")
                # split the x load by pixel range so the first chunks'
                # transposes can start after a fraction of the transfer
                nc.sync.dma_start(
                    xb[:, :, 0:N4],
                    x[b, :, 0:N4].rearrange("(a p) n -> p a n", p=P),
                )
                nc.sync.dma_start(
                    xb[:, :, N4:N8],
                    x[b, :, N4:N8].rearrange("(a p) n -> p a n", p=P),
                )
                nc.sync.dma_start(
                    xb[:, :, N8:N],
                    x[b, :, N8:N].rearrange("(a p) n -> p a n", p=P),
                )

                xt = xt_pool.tile([P, NJ, DP2], BF16, tag="xt")
                ss = smalls.tile([P, NJ], F32, tag="ss")
                nc.gpsimd.memset(ss, 1.0)

                # warm 1: observes only the xb DMA semaphore.
                warm = nc.tensor.matmul(
                    lg[0:2, par, 1020:1022],
                    xb[:, 0, 0:2],
                    xb[:, 0, 0:2],
                    start=True,
                    stop=True,
                    skip_group_check=True,
                )
                if "last_pe" in state:
                    add_dep_helper(
                        warm.ins,
                        state["last_pe"].ins,
                        sync=False,
                        reason="pin batch warm after prior PE work",
                    )
                # (no warm for the logits-bank WAR: the prescale of batch
                # b-2 is this bank's only reader, so the first logits
                # matmul legally carries that single semaphore itself)
                prev_pe = warm
                for j, (n0, nj) in enumerate(NCHUNKS):
                    r = j % 2
                    if j in (2, 4):
                        # warms 3/4: observe only the second/third x DMA
                        # part, so the next chunk's first transpose carries
                        # only its region WAR sem (S3_LW allows one sync
                        # wait per Matmult).
                        npart = N4 if j == 2 else N8
                        warm3 = nc.tensor.matmul(
                            lg[0:2, par, 1022:1024],
                            xb[:, 0, npart : npart + 2],
                            xb[:, 0, npart : npart + 2],
                            start=True,
                            stop=True,
                            skip_group_check=True,
                        )
                        add_dep_helper(
                            warm3.ins,
                            prev_pe.ins,
                            sync=False,
                            reason="pin DMA-part warm after prior PE work",
                        )
                    for a in range(DCH):
                        nc.tensor.transpose(
                            xtp[:nj, r, a * P : (a + 1) * P].bitcast(F32R),
                            xb[:, a, n0 : n0 + nj],
                            ident,
                        )
                        prev_pe = nc.tensor.matmul(
                            lg[:nj, par, j * K : (j + 1) * K],
                            xb[:, a, n0 : n0 + nj],
                            wt_sb[:, a, :],
                            start=(a == 0),
                            stop=(a == DCH - 1),
                            skip_group_check=True,
                        )
                    # PSUM -> SBUF bf16 eviction
                    if CAST_ENG[j] == "a":
                        nc.scalar.copy(xt[:nj, j, 0:D], xtp[:nj, r])
                    else:
                        nc.vector.tensor_copy(xt[:nj, j, 0:D], xtp[:nj, r])

                    def do_square(jq):
                        n0q, njq = NCHUNKS[jq]
                        if SQ_ENG[jq] == "v":
                            nc.vector.scalar_tensor_tensor(
                                out=sqj[:njq],
                                in0=xt[:njq, jq, 0:D],
                                scalar=1.0,
                                in1=xt[:njq, jq, 0:D],
                                op0=ALU.mult,
                                op1=ALU.mult,
                                accum_out=ss[:njq, jq : jq + 1],
                            )
                        else:
                            nc.scalar.activation(
                                sqj2[:njq],
                                xt[:njq, jq, 0:D],
                                ACTF.Square,
                                accum_out=ss[:njq, jq : jq + 1],
                            )

                    # squares of chunks 0-3 are deferred to iterations 6-9:
                    # they feed only the NEXT batch's tail, and the early
                    # iterations must keep the casts at the queue heads so
                    # the PE's 2-deep region rotation never starves.  The
                    # last batch's tail is on the drain critical path, so
                    # its squares run immediately instead.
                    if b == BPC - 1:
                        do_square(j)
                    else:
                        if j >= 4:
                            do_square(j)
                        if j >= 6:
                            do_square(j - 6)
                    # deferred tail/epilog pieces of earlier batches (the
                    # phase2 negd read of vl must be emitted before the
                    # first aggregation matmul overwrites vl)
                    for f in fillers.get(j, ()):
                        f()
                    # interleave the previous batch's aggregation matmuls;
                    # starts at j=3, right after its atp filler, so the PE
                    # has matmul work in nearly every chunk slot
                    if b > 0:
                        agg_chunks(b - 1, AGG_SCHED.get(j, ()))

                state[b] = {"xt": xt, "ss": ss, "par": par}

            def agg_chunks(b, js):
                if not js:
                    return
                st = state[b]
                xt, atp = st["xt"], st["atp"]
                for j in js:
                    n0, nj = NCHUNKS[j]
                    nc.tensor.matmul(
                        vl,
                        atp[:nj, j],
                        xt[:nj, j, 0:D],
                        start=(j == 0),
                        stop=(j == NJ - 1),
                    )
                    last = nc.tensor.matmul(
                        asum,
                        atp[:nj, j],
                        xt[:nj, j, D : D + 2],
                        start=(j == 0),
                        stop=(j == NJ - 1),
                    )
                    if j == NJ - 1:
                        state["last_pe"] = last

            for b in range(BPC):
                fillers = {}
                if b > 0:
                    t = tail_pieces(b - 1)
                    fillers.setdefault(0, []).extend([t[0], t[1]])
                    fillers.setdefault(2, []).extend([t[2], t[3]])
                    fillers.setdefault(3, []).append(t[4])
                if b > 1:
                    p = phase2_pieces(b - 2)
                    # negd must precede the first aggregation matmul (vl
                    # WAR); the rest go late so they don't congest ACT/DVE
                    # while the early-chunk PSUM evictions are queued.
                    fillers.setdefault(2, []).append(p[0])
                    fillers.setdefault(6, []).append(p[1])
                    fillers.setdefault(7, []).append(p[2])
                    fillers.setdefault(8, []).append(p[3])
                phase1(b, fillers)
            # drain: tail of the last batch, epilog of the last two
            for f in tail_pieces(BPC - 1):
                f()
            for f in phase2_pieces(BPC - 2):
                f()
            agg_chunks(BPC - 1, list(range(NJ)))
            for f in phase2_pieces(BPC - 1):
                f()

    return nc


_NC = None


def _patch_act_tables():
    """Force every ScalarE activation onto the one table set that contains
    {copy, square, ln, exp} so the kernel pays a single ACT_TABLE_LOAD
    instead of thrashing between exp_and_others and natural_log."""
    import concourse.bacc as _bacc_mod
    orig = _bacc_mod.get_activation_tables

    def patched(arch):
        tables = dict(orig(arch))
        assert "natural_log_exp_and_others" in tables
        return {
            name: (funcs if name == "natural_log_exp_and_others" else set())
            for name, funcs in tables.items()
        }

    _bacc_mod.get_activation_tables = patched


def _get_nc():
    global _NC
    if _NC is None:
        _patch_act_tables()
        nc = bacc.Bacc("TRN2", target_bir_lowering=False)
        _emit(nc)
        nc.compile()
        _NC = nc
    return _NC


def _make_in_maps(x, conv_w, centroids):
    B = x.shape[0]
    xs = np.ascontiguousarray(x, dtype=np.float32).reshape(B, D, N)
    wt = np.ascontiguousarray(conv_w.T, dtype=np.float32)
    cent = np.ascontiguousarray(centroids, dtype=np.float32)
    in_maps = []
    for c in range(8):
        in_maps.append(
            {
                "x": np.ascontiguousarray(xs[c * BPC : (c + 1) * BPC]),
                "wt": wt,
                "cent": cent,
            }
        )
    return in_maps


def _run(x, conv_w, centroids, trace=False):
    nc = _get_nc()
    res = run_bass_kernel_spmd(
        nc,
        _make_in_maps(x, conv_w, centroids),
        core_ids=list(range(8)),
        trace=trace,
    )
    outs = [r["out"].reshape(BPC, K * D) for r in res.results]
    full = np.concatenate(outs, axis=0)
    return full, res


def kernel(x, conv_w, centroids):
    full, _ = _run(x, conv_w, centroids, trace=False)
    return full

